# revision 70
# baseline (speedup 1.0000x reference)
"""Causal single-head attention (B=4, T=4096, C=1024, D=64) on 8 NeuronCores.

Sharding: core c = (batch b = c % 4, half h = c // 4).
Each core handles ALL queries of its batch, but only its half of the key
blocks (256-token key blocks with block index ≡ h mod 2).  Identical program
on every core (pure SPMD); cores differ only in input data (the mask blob
and an h selector read into a PE register at runtime).  Each core emits
unnormalized partials U^T = [V|1]^T @ exp(S^T) per query supertile; host
combines: O = (U0 + U1)[:64] / (U0 + U1)[64].

All bf16 (fp8 fails the 2e-2 gate: multiplicative weight noise passes
straight through softmax, ~3% per fp8 source).  Dataflow:
  xqs [128, ctile, block, half, 256]: x^T with each 512-token group split
        into its even/odd 256-token key blocks; ONE tensor feeds Q (static
        contiguous slices) and K/V (half picked via ds(h-register) in the
        matmul rhs — no separately-gathered key tensor, 8.4MB total DMA)
  Q^T   = col-tiled matmuls ([Wq|Wq] stationary, tile_position (0,0)/(0,64)
          process two 512-query chunks concurrently), then copied to both
          partition halves of qT2 [128, T] (DVE 4x-mode dup)
  K^T/V^T = packed [Wk|Wv] matmul (M=128): K rows 0:64, V rows 64:128 of
          PSUM; K tiles -> kT2 [128, 8, 128] (even local tile rows 0:64,
          odd rows 64:128), V -> vT [65, TK] (row 64 = ones)
  V'    = PE transpose of vT per 128-key tile -> vP [128, 16, 65]
  S^T   = TWO concurrent row-tiled matmuls per key-tile pair (tile_position
          (0,0)/(64,0), contraction D=64 each) -> s2 [128, 2, 512] f32 PSUM
  P     = ACT exp(s2/8) -> bf16; diagonal pair masked on DVE (4x mode)
  U^T  += V'_j^T @ P_j per key tile (M=65 incl. ones row for denominators)
Schedule: software-pipelined — supertile st's score/exp stream interleaves
with st-1's PV matmuls (strict in-order PE FIFO never head-blocks on the
exp-paced PSUM buffers); projection chunks slot between supertiles; the
last supertile runs its masked diagonal pair first so the kernel tail is a
bare exp->PV->copy->DMA chain.
"""
import sys
import numpy as np
import ml_dtypes

if "/opt/trn_rl_repo" not in sys.path:
    sys.path.insert(0, "/opt/trn_rl_repo")

import concourse.bacc as bacc
import concourse.mybir as mybir
from concourse import tile
from concourse import bass_utils
from concourse.bass import ds

bf16 = mybir.dt.bfloat16
f32 = mybir.dt.float32
i32 = mybir.dt.int32
BF = ml_dtypes.bfloat16

B, T, C, D = 4, 4096, 1024, 64
NST = 8          # query supertiles per batch (512 queries each)
STQ = 512
TK = T // 2      # key tokens per core
NKT = TK // 128  # local 128-key tiles per core (16)
NCT = C // 128   # contraction c-tiles (8)

# const blob byte offsets (per partition): msk | wqq | wkv | idn
_MSK0, _WQ0, _WKV0, _IDN0, _H0, _CEND = 0, 2048, 4096, 6144, 6280, 6284

_CACHE = {}


def _build():
    nc = bacc.Bacc(None, target_bir_lowering=False, debug=False, num_devices=8)

    xqs = nc.dram_tensor("xqs", [128, NCT, NKT // 2, 2, 256], bf16,
                         kind="ExternalInput")
    cst = nc.dram_tensor("cst", [128, _CEND], mybir.dt.uint8,
                         kind="ExternalInput")
    out = nc.dram_tensor("out", [65, T], f32, kind="ExternalOutput")

    with tile.TileContext(nc) as tc:
        with tc.tile_pool(name="sb", bufs=1) as sb, \
             tc.tile_pool(name="pp", bufs=3) as pp, \
             tc.tile_pool(name="ps", bufs=2, space="PSUM") as ps:

            # ---- resident inputs ----
            xqs_sb = sb.tile([128, NCT, NKT // 2, 2, 256], bf16, tag="xqs")
            cst_sb = sb.tile([128, _CEND], mybir.dt.uint8, tag="cst")
            msk_sb = cst_sb[:, _MSK0:_WQ0].bitcast(bf16).rearrange(
                "p (two q) -> p two q", two=2)
            wqq_sb = cst_sb[:, _WQ0:_WKV0].bitcast(bf16).rearrange(
                "p (t m) -> p t m", t=NCT)
            wkv_sb = cst_sb[:, _WKV0:_IDN0].bitcast(bf16).rearrange(
                "p (t m) -> p t m", t=NCT)
            idn_sb = cst_sb[0:65, _IDN0:_IDN0 + 130].bitcast(bf16)

            def dma_chunk(c, tsplit=1):
                # chunk c = key cols [256c, 256c+256) of both halves =
                # query tokens [512c, 512c+512); tsplit splits along the
                # contraction dim so the first matmuls start sooner
                tn = NCT // tsplit
                for s in range(tsplit):
                    nc.sync.dma_start(
                        xqs_sb[:, tn * s:tn * (s + 1), c, :, :],
                        xqs[:, tn * s:tn * (s + 1), c, :, :])

            # critical-path data first: weights/idn (idn gates the PE-FIFO
            # transpose chain!), first chunks; masks can come later
            nc.sync.dma_start(cst_sb[:, _WQ0:], cst[:, _WQ0:])
            dma_chunk(0, tsplit=4)
            dma_chunk(1, tsplit=2)
            nc.sync.dma_start(cst_sb[:, 0:_WQ0], cst[:, 0:_WQ0])
            for c in range(2, 8):
                dma_chunk(c)

            # ---- persistent intermediates ----
            qT2 = sb.tile([128, T], bf16, tag="qT2")   # rows 64:128 duplicate
            kT2 = sb.tile([128, NKT // 2, 128], bf16, tag="kT2")
            vT = sb.tile([65, TK], bf16, tag="vT")     # row 64 = ones
            vP = sb.tile([128, NKT, 65], bf16, tag="vP")

            nc.gpsimd.memset(vT[64:65, :], 1.0)

            # warm the ACT exp table during the DMA ramp (1.28us load)
            scr = sb.tile([1, 2], f32, tag="scr")
            nc.vector.memset(scr[:], 0.0)
            nc.scalar.activation(scr[0:1, 1:2], scr[0:1, 0:1],
                                 mybir.ActivationFunctionType.Exp, scale=1.0)

            # h selector register (PE reads it in K/V matmul rhs APs)
            hreg = nc.tensor.alloc_register("hreg")
            nc.tensor.reg_load(hreg, cst_sb[0:1, _H0:_CEND].bitcast(i32))
            hsv = nc.tensor.snap(hreg, donate=True, min_val=0, max_val=1)

            def kv_chunk(c0, cols):
                sl = slice(c0, c0 + cols)
                acc = ps.tile([128, 512], f32, tag="work", name=f"kv{c0}",
                              bufs=2)
                l0 = c0 // 256
                for t in range(NCT):
                    nc.tensor.matmul(acc[:, 0:cols], wkv_sb[:, t, :],
                                     xqs_sb[:, t, l0:l0 + cols // 256,
                                            ds(hsv, 1), :],
                                     start=(t == 0), stop=(t == NCT - 1))
                nc.vector.tensor_copy(vT[0:64, sl], acc[64:128, 0:cols])
                for i in range(cols // 128):
                    tau = c0 // 128 + i
                    tp = ps.tile([128, 65], bf16, tag="work", name=f"tp{tau}",
                                 bufs=2)
                    nc.tensor.transpose(tp[:], vT[:, 128 * tau:128 * (tau + 1)],
                                        idn_sb[:])
                    nc.vector.tensor_copy(vP[:, tau, :], tp[:])
                for i in range(cols // 128):
                    tau = c0 // 128 + i        # local key tile
                    pj, po = tau // 2, tau % 2
                    nc.vector.tensor_copy(
                        kT2[64 * po:64 * po + 64, pj, :],
                        acc[0:64, 128 * i:128 * (i + 1)])

            def q_chunks(q0, nq):
                """nq=1: plain M=64 proj of chunk q0 (fast ramp path).
                nq=2: col-tiled pair — chunks q0 (cols 0:64 of the array)
                and q0+1 (cols 64:128) run concurrently on real HW."""
                acc = ps.tile([128, 512], f32, tag="work", name=f"q{q0}",
                              bufs=2)
                for t in range(NCT):
                    for d in range(nq):
                        nc.tensor.matmul(acc[64 * d:64 * d + 64, :],
                                         wqq_sb[:, t, 64 * d:64 * d + 64],
                                         xqs_sb[:, t, q0 + d, :, :],
                                         start=(t == 0), stop=(t == NCT - 1))
                for d in range(nq):
                    sl = slice(512 * (q0 + d), 512 * (q0 + d + 1))
                    nc.vector.tensor_copy(qT2[0:64, sl],
                                          acc[64 * d:64 * d + 64, :])
                # duplicate into rows 64:128 (DVE 4x mode, SBUF->SBUF bf16)
                dsl = slice(512 * q0, 512 * (q0 + nq))
                nc.vector.tensor_copy(qT2[64:128, dsl], qT2[0:64, dsl])

            def att_sc(st, pj):
                """Scores pair pj of supertile st -> exp -> (mask); returns
                the bf16 P tile for the deferred PV stage."""
                qsl = slice(STQ * st, STQ * (st + 1))
                s2 = ps.tile([128, 2, STQ], f32, tag="s", name=f"s{st}_{pj}")
                p2 = pp.tile([128, 2, STQ], bf16, tag="p", name=f"p{st}_{pj}",
                             bufs=16)
                nc.tensor.matmul(s2[:, 0, :], kT2[0:64, pj, :],
                                 qT2[0:64, qsl], start=True, stop=True)
                nc.tensor.matmul(s2[:, 1, :], kT2[64:128, pj, :],
                                 qT2[64:128, qsl], start=True, stop=True)
                nc.scalar.activation(p2[:], s2[:],
                                     mybir.ActivationFunctionType.Exp,
                                     scale=0.125)
                if pj == st:            # diagonal pair -> causal masks
                    nc.vector.tensor_mul(p2[:], p2[:], msk_sb[:])
                return p2

            def att_pv(st, pj, p2, u, first=None, last=None):
                if first is None:
                    first, last = pj == 0, pj == st
                nc.tensor.matmul(u[:], vP[:, 2 * pj, :], p2[:, 0, :],
                                 start=first, stop=False)
                nc.tensor.matmul(u[:], vP[:, 2 * pj + 1, :], p2[:, 1, :],
                                 start=False, stop=last)

            def att_out(st, u):
                qsl = slice(STQ * st, STQ * (st + 1))
                u_sb = pp.tile([65, STQ], f32, tag="u_sb", name=f"us{st}")
                nc.vector.tensor_copy(u_sb[:], u[:])
                nc.sync.dma_start(out[:, qsl], u_sb[:])

            # ---- software-pipelined schedule ----
            # Supertile st's score pairs interleave with st-1's PV pairs so
            # the PE always has ready work while ACT drains exps; proj
            # chunks (q/kv) slot between supertiles.  st=7 inlines its PV
            # after each exp to keep the kernel tail short.
            def fill(st):            # q-proj issued between supertiles
                if st == 1:
                    q_chunks(1, 1)
                elif st == 2:
                    q_chunks(2, 2)
                elif st == 3:
                    q_chunks(4, 2)
                elif st == 5:
                    q_chunks(6, 2)

            def fill_mid(st):        # kv chunks issued AFTER st's first
                if st == 1:          # score pair (only the last pairs of
                    kv_chunk(256, 256)     # a supertile consume them), so
                elif st == 2:              # ACT's exp stream starts first
                    kv_chunk(512, 512)
                elif st == 4:
                    kv_chunk(1024, 512)
                elif st == 6:
                    kv_chunk(1536, 512)

            q_chunks(0, 1)
            kv_chunk(0, 256)       # pair 0

            prev = None            # (st, u, [p2 ...]) awaiting PV
            for st in range(NST):
                fill(st)
                u = ps.tile([65, STQ], f32, tag="u", name=f"u{st}", bufs=2)
                plist = []
                # last supertile: diagonal (masked) pair first so the
                # kernel tail is a plain exp->PV, no mask in the chain
                order = ([st] + list(range(st))) if st == NST - 1                     else list(range(st + 1))
                for i, pj in enumerate(order):
                    p2 = att_sc(st, pj)
                    plist.append(p2)
                    if i == 0:
                        fill_mid(st)
                    if st == NST - 1:
                        att_pv(st, pj, p2, u, first=(i == 0),
                               last=(i == st))    # inline on the last st
                    if prev is not None and i < prev[0] + 1:
                        att_pv(prev[0], i, prev[2][i], prev[1])
                        if i == prev[0]:
                            att_out(prev[0], prev[1])
                if st < NST - 1:
                    prev = (st, u, plist)
            att_out(NST - 1, u)

    nc.compile()
    return nc


def _get_nc():
    if "nc" not in _CACHE:
        _CACHE["nc"] = _build()
    return _CACHE["nc"]


def _pack_ct(a):
    """[C, N] -> [128, NCT, N] with channel c = 128*t + p."""
    n = a.shape[1]
    return np.ascontiguousarray(
        a.reshape(NCT, 128, n).transpose(1, 0, 2))


def kernel(x, Wq, Wk, Wv, _trace=False):
    x = np.asarray(x)
    nc = _get_nc()

    wqq = np.concatenate([np.asarray(Wq), np.asarray(Wq)], axis=1).astype(BF)
    wkv = np.concatenate([np.asarray(Wk), np.asarray(Wv)], axis=1).astype(BF)
    wqq_h = _pack_ct(wqq)                              # [128, 8, 128]
    wkv_h = _pack_ct(wkv)
    idn = np.eye(65, dtype=BF)

    j = np.arange(128)[:, None]
    i = np.arange(STQ)[None, :]
    csts = {}
    for h in range(2):
        m0 = (j <= i - 256 * h).astype(BF)
        m1 = (j <= i - 256 * h - 128).astype(BF)
        mh = np.stack([m0, m1], axis=1)                # [128, 2, 512]
        cst = np.zeros((128, _CEND), np.uint8)
        cst[:, _MSK0:_WQ0] = mh.reshape(128, 1024).view(np.uint8)
        cst[:, _WQ0:_WKV0] = wqq_h.reshape(128, 1024).view(np.uint8)
        cst[:, _WKV0:_IDN0] = wkv_h.reshape(128, 1024).view(np.uint8)
        cst[0:65, _IDN0:_IDN0 + 130] = idn.view(np.uint8).reshape(65, 130)
        cst[:, _H0:_CEND] = np.full((128, 1), h, np.int32).view(np.uint8)
        csts[h] = cst

    xqs_h = []
    for b in range(B):
        xT = np.ascontiguousarray(x[b].T).astype(BF)   # [C, T]
        # token 512*l + 256*e + r -> xqs[., t, l, e, r]
        xqs = _pack_ct(xT.view(np.uint16)).reshape(
            128, NCT, NKT // 2, 2, 256)
        xqs_h.append(xqs.view(BF))

    in_maps = []
    for c in range(8):
        b, h = c % 4, c // 4
        in_maps.append({"xqs": xqs_h[b], "cst": csts[h]})

    res = bass_utils.run_bass_kernel_spmd(nc, in_maps, core_ids=list(range(8)),
                                          trace=_trace)
    _CACHE["last_results"] = res

    O = np.empty((B, T, D), dtype=np.float32)
    for b in range(B):
        U = res.results[b]["out"] + res.results[b + 4]["out"]    # [65, T]
        O[b] = (U[:D] / U[D:D + 1]).T
    return O
